# revision 2
# baseline (speedup 1.0000x reference)
"""Trainium2 Bass kernel for nn_DistillingLayer: per-channel shared-weight
Conv1d(k=3, stride=2, pad=1) + ELU + MaxPool1d(k=3, stride=2, pad=1) over
x:(16, 4096, 512) f32 -> out:(16, 1024, 512) f32.

Strategy (v2)
-------------
- Shard L across the 8 cores: core k owns x[:, 512k:512(k+1), :] for ALL 16
  batches (plus a 3-row left halo). Partition p = 16c + b owns 64 consecutive
  L-rows of batch b (c-th 64-row chunk of the core's 512-row slab), so each
  partition's input is one contiguous HBM run and the whole conv+pool stays
  per-partition local.
- One persistent SBUF buffer X[128, 67 rows] is filled progressively by 7
  chunked input DMAs (no waits between them -> the SWDGE ring streams the
  full 17.6 MB at HBM rate). Consecutive tiles' conv windows overlap chunk
  boundaries; only the per-partition 3-row halo is re-read from HBM once
  (4.7% instead of the 18.75% per-tile halo of a batch-sharded layout).
- Input DMAs cast f32->bf16 in flight (SWDGE feature): halves SBUF traffic
  and unlocks the DVE 2x_1p 16-bit mode for the conv/pool tensor ops.
- ELU is monotonic, so maxpool commutes with it: pool the pre-activation
  conv outputs, then ELU once on the pooled rows (1/2 the rows of conv).
- Engine split per tile: ACT does conv tap0 (+bias) and the ELU Relu/Exp;
  DVE does conv taps 1/2 (scalar_tensor_tensor), the two pool maxes and the
  final (e-1) max v. GpSimd only generates input-DMA descriptors.
- Output stores go on the idle sync (SP) HWDGE ring, so their
  wait-for-compute never blocks input streaming (separate ring from SWDGE).
- The left conv boundary (core 0 / global rows -3..-1) is handled with host
  data instead of a core-dependent program: pad row values are chosen so the
  out-of-range conv row c[-1] evaluates to ~-1e32 and loses every pool max,
  reproducing the reference's -inf pool padding while keeping SPMD uniform.
- Weights/bias are baked as immediates; the compiled module is cached per
  (w, b) value.

Toolchain workaround (see inline comment): a BIR post-pass splits
multi-wait instructions - this walrus build allows one sync wait per
instruction.
"""

import json as _json
import os
import sys

import numpy as np

for _p in ("/opt/trn_rl_repo", "/root/.axon_site/_ro/trn_rl_repo"):
    if os.path.isdir(_p) and _p not in sys.path:
        sys.path.append(_p)

import concourse.bass as bass
import concourse.bass2jax as bass2jax
import concourse.bass_utils as bass_utils
import concourse.mybir as mybir
from concourse.bass_utils import run_bass_kernel_spmd
from concourse.tile import TileContext

# ---------------------------------------------------------------------------
# REQUIRED workaround: this container's walrus build rejects instructions
# carrying more than one sync wait ("Too many sync wait commands" in
# setupSyncWait). Tile's scheduler freely attaches several waits to one
# instruction, so post-process the BIR JSON before compile: hoist all but the
# last wait onto same-engine NoOps inserted just before the instruction
# (per-engine program order makes sequential waits equivalent to a
# multi-wait).
# ---------------------------------------------------------------------------

_orig_compile_bir_kernel = bass_utils.compile_bir_kernel


def _split_multi_waits(bir_json: bytes) -> bytes:
    j = _json.loads(bir_json)
    ctr = 0
    changed = False
    for fn in j["functions"]:
        for bb in fn["blocks"]:
            out = []
            for ins in bb["instructions"]:
                si = ins.get("sync_info")
                waits = (si.get("on_wait") or []) if si else []
                if len(waits) > 1:
                    changed = True
                    for w in waits[:-1]:
                        ctr += 1
                        out.append(
                            {
                                "debug": ins.get("debug", 0),
                                "engine": ins["engine"],
                                "ins": [],
                                "outs": [],
                                "name": f"waitsplit-{ctr}",
                                "opcode": "NoOp",
                                "text_hint": "waitsplit",
                                "sync_info": {"on_update": [], "on_wait": [w]},
                            }
                        )
                    si["on_wait"] = [waits[-1]]
                out.append(ins)
            bb["instructions"] = out
    if not changed:
        return bir_json
    return _json.dumps(j).encode()


def _patched_compile_bir_kernel(bir_json, tmpdir, neff_name="file.neff"):
    return _orig_compile_bir_kernel(_split_multi_waits(bir_json), tmpdir, neff_name)


bass_utils.compile_bir_kernel = _patched_compile_bir_kernel
bass2jax.compile_bir_kernel = _patched_compile_bir_kernel

# The first TileContext exit barrier's per-engine drains are redundant (the
# tail waits already cover all completions); use the cheap sequencer-level
# variant there. The SECOND barrier stays full — its drains restore
# engine/queue state so the loaded NEFF can re-execute.
try:
    from concourse.vector_clock import ScopedClock as _ScopedClock

    def _tail_drain_and_barrier(self, tick_clock, wait_clock):
        drain_inst = self.nc.sync.drain()
        wait_clock.add_sem_waits(
            drain_inst.ins, _ScopedClock({None: tick_clock.global_clock})
        )
        self.nc.all_engine_barrier(sem_only=True)
        assert self.sems is not None
        popped = self.nc._tile_sem_poison_stack.pop()
        assert popped is self._sem_poison
        self.nc.clear_and_free_semaphores(list(self.sems.allocated().values()))
        self.nc.all_engine_barrier()

    TileContext._drain_and_barrier = _tail_drain_and_barrier
except Exception:
    pass

# ---------------------------------------------------------------------------

N_CORES = 8
B, L, D = 16, 4096, 512
SLAB = L // N_CORES          # 512 x-rows per core
RPP = SLAB * B // 128        # 64 x-rows per partition
XROWS = RPP + 3              # 67 (3-row left halo + 64 own rows)
SLABP = SLAB + 3             # per-core DRAM slab rows (incl. halo)
OPP = RPP // 4               # 16 pool-output rows per partition
OROWS = L // 4 // N_CORES    # 128 pool rows per core

F32 = mybir.dt.float32
BF16 = mybir.dt.bfloat16
ALU = mybir.AluOpType
AF = mybir.ActivationFunctionType

# (x_row_start, St): tile t computes pool rows [s/4, (s+St)/4) per partition
# from X rows [s, s+St+3). Small head tiles start compute early; small tail
# tiles shorten the post-DMA dependency chain.
TILES = [(0, 4), (4, 4), (8, 16), (24, 16), (40, 16), (56, 4), (60, 4)]
# (X_row_start, rows) per input DMA chunk; chunk 0 includes the 3 halo rows.
CHUNKS = [(0, 7), (7, 4), (11, 16), (27, 16), (43, 16), (59, 4), (63, 4)]

_cache: dict = {}

# Exposed for test harnesses: the BassKernelResults of the last run.
LAST_RESULT = None


def _build(w0: float, w1: float, w2: float, bias: float) -> bass.Bass:
    nc = bass.Bass()
    x = nc.dram_tensor("x", [B, SLABP, D], F32, kind="ExternalInput")
    y = nc.dram_tensor("y", [B, OROWS, D], F32, kind="ExternalOutput")

    with TileContext(nc) as tc:
        with (
            tc.tile_pool(name="xp", bufs=1) as xp,
            tc.tile_pool(name="yp", bufs=2) as yp,
            tc.tile_pool(name="pp", bufs=2) as pp,
            tc.tile_pool(name="rp", bufs=2) as rp,
        ):
            X = xp.tile([128, XROWS * D], BF16)

            # Stream the whole slab in upfront: the persistent X buffer is
            # written once and never recycled, so none of these DMAs carries
            # a wait — the SWDGE ring drains them back-to-back at HBM rate.
            # DRAM AP dims: [chunk c (8), batch b (16), row-run] -> partition
            # p = 16c + b; each partition's run is one contiguous HBM read.
            for rs, rn in CHUNKS:
                nc.gpsimd.dma_start(
                    out=X[:, rs * D : (rs + rn) * D],
                    in_=bass.AP(
                        x,
                        rs * D,
                        [[RPP * D, 8], [SLABP * D, 16], [1, rn * D]],
                    ),
                )

            def conv(t):
                s, St = TILES[t]
                Q = St // 2 + 1
                Y = yp.tile([128, Q * D], BF16)
                Xv = X[:, s * D : (s + St + 3) * D].rearrange(
                    "p (r d) -> p r d", d=D
                )
                y3 = Y[:, :].rearrange("p (q d) -> p q d", d=D)
                ya = Xv[:, 0 : 2 * Q - 1 : 2, :]
                yb = Xv[:, 1 : 2 * Q : 2, :]
                yc = Xv[:, 2 : 2 * Q + 1 : 2, :]
                ys = y3[:, 0:Q, :]
                nc.scalar.activation(ys, ya, AF.Copy, bias=bias, scale=w0)
                nc.vector.scalar_tensor_tensor(
                    ys, yb, w1, ys, op0=ALU.mult, op1=ALU.add
                )
                nc.vector.scalar_tensor_tensor(
                    ys, yc, w2, ys, op0=ALU.mult, op1=ALU.add
                )
                return Y

            def pool_elu_store(t, Y):
                s, St = TILES[t]
                Jt = St // 4
                y3 = Y[:, :].rearrange("p (q d) -> p q d", d=D)
                P = pp.tile([128, Jt * D], BF16)
                R = rp.tile([128, Jt * D], F32)
                p3 = P[:, :].rearrange("p (j d) -> p j d", d=D)
                nc.vector.tensor_tensor(
                    p3,
                    y3[:, 0 : 2 * Jt - 1 : 2, :],
                    y3[:, 1 : 2 * Jt : 2, :],
                    op=ALU.max,
                )
                nc.vector.tensor_tensor(
                    p3, p3, y3[:, 2 : 2 * Jt + 1 : 2, :], op=ALU.max
                )
                # ELU(v) = max(v, exp(min(v,0)) - 1)
                nc.scalar.activation(R[:, :], P[:, :], AF.Relu, scale=-1.0)
                nc.scalar.activation(R[:, :], R[:, :], AF.Exp, scale=-1.0)
                nc.vector.scalar_tensor_tensor(
                    R[:, :], R[:, :], -1.0, P[:, :], op0=ALU.add, op1=ALU.max
                )
                nc.sync.dma_start(
                    out=bass.AP(
                        y,
                        (s // 4) * D,
                        [[OPP * D, 8], [OROWS * D, 16], [1, Jt * D]],
                    ),
                    in_=R[:, :],
                )

            # Skew pool/ELU one tile behind conv so no engine stalls on a
            # same-tile cross-engine dependency.
            pend = None
            for t in range(len(TILES)):
                Yt = conv(t)
                if pend is not None:
                    pool_elu_store(*pend)
                pend = (t, Yt)
            pool_elu_store(*pend)
    return nc


def kernel(x: np.ndarray, w: np.ndarray, b: np.ndarray) -> np.ndarray:
    global LAST_RESULT
    w = np.asarray(w, dtype=np.float32)
    bb = np.asarray(b, dtype=np.float32)
    key = (float(w[0]), float(w[1]), float(w[2]), float(bb[0]))
    if key not in _cache:
        _cache[key] = _build(*key)
    nc = _cache[key]

    x = np.asarray(x, dtype=np.float32)
    assert x.shape == (B, L, D), x.shape
    xpad = np.empty((B, L + 3, D), dtype=np.float32)
    xpad[:, 3:] = x
    # Left-edge pad rows (seen only by core 0): row 2 (= x[-1]) must be an
    # exact conv zero-pad; rows 0-1 are free, so pick them to drive the
    # out-of-range conv row c[-1] to ~-1e32 — it then loses every pool max,
    # matching the reference's -inf pool padding without a core-special
    # program.
    w0, w1 = float(w[0]), float(w[1])
    r0 = r1 = 0.0
    t = -1e32
    if abs(w0) >= abs(w1) and w0 != 0.0:
        r0 = float(np.clip(t / w0, -3e38, 3e38))
    elif w1 != 0.0:
        r1 = float(np.clip(t / w1, -3e38, 3e38))
    xpad[:, 0] = r0
    xpad[:, 1] = r1
    xpad[:, 2] = 0.0

    in_maps = [
        {"x": np.ascontiguousarray(xpad[:, SLAB * k : SLAB * k + SLABP])}
        for k in range(N_CORES)
    ]
    res = run_bass_kernel_spmd(nc, in_maps, core_ids=list(range(N_CORES)))
    LAST_RESULT = res
    return np.concatenate([r["y"] for r in res.results], axis=1)
